# revision 21
# baseline (speedup 1.0000x reference)
"""Dense dot-product attention (B=32, S=2048, D=128, fp32) on 8 TRN2 cores.

Sharding: batch dim B=32 split across 8 cores (4 batches/core); each core
computes full S x S attention for its batches independently (no collectives).
Host-side shard prep feeds Q,K pre-transposed ([D,S] per batch) and the
device returns O^T ([D,S]); the gather step transposes back. All matmuls run
in fp32r (tf32-like, ~12 mantissa bits).

Per-core kernel, per batch ("S^T layout", k on partitions):
  for each q-phase (1024 wide), for each k-chunk j (16 x 128):
    S^T_j = Kt_j.T @ Qt[:, phase]      (PE, fp32r -> PSUM fp32)
    P^T_j = exp(scale * S^T_j)         (ACT, PSUM -> SBUF fp32r)
    l    += ones.T @ P^T_j             (PE, row sums in PSUM [1, q])
    O^T  += V_j.T @ P^T_j              (PE, PSUM [128d, q])
  drain: O^T -> SBUF; 1/l (DVE fast reciprocal); broadcast 1/l across
  partitions via a DRAM round-trip; O^T * (1/l) -> DMA out.
"""

import sys

if "/opt/trn_rl_repo" not in sys.path:
    sys.path.insert(0, "/opt/trn_rl_repo")

import numpy as np

import concourse.bacc as bacc
import concourse.mybir as mybir
import concourse.tile as tile
from concourse import bass_utils

N_CORES = 8
B = 32
S = 2048
D = 128
P = 128
BPC = B // N_CORES          # batches per core = 4
NJ = S // P                 # 16 k-chunks of 128
QH = 1024                   # q-phase width
NPH = S // QH               # 2 phases
NC_ = 512                   # matmul moving-operand chunk (fp32 max)
SCALE = 1.0 / float(np.sqrt(D))

f32 = mybir.dt.float32
f32r = mybir.dt.float32r
EXP = mybir.ActivationFunctionType.Exp


HWDGE_LOADS = True


def build(repeat=1):
    """repeat>1 duplicates the whole per-core workload (same inputs/outputs)
    back-to-back inside one NEFF — used only for differential wall-clock
    timing of the hardware kernel (host/dispatch overhead cancels)."""
    nc = bacc.Bacc("TRN2", target_bir_lowering=False, debug=False)

    Qtd = nc.dram_tensor("Qt", [BPC, D, S], f32, kind="ExternalInput")
    Ktd = nc.dram_tensor("Kt", [BPC, D, S], f32, kind="ExternalInput")
    Vd = nc.dram_tensor("V_p", [BPC, S, D], f32, kind="ExternalInput")
    Otd = nc.dram_tensor("Ot", [BPC, D, S], f32, kind="ExternalOutput")

    with tile.TileContext(nc) as tc:
        with (
            tc.tile_pool(name="const", bufs=1) as const_pool,
            tc.tile_pool(name="inp", bufs=2) as in_pool,
            tc.tile_pool(name="pt", bufs=6) as pt_pool,
            tc.tile_pool(name="misc", bufs=2) as misc_pool,
            tc.tile_pool(name="ot", bufs=2) as ot_pool,
            tc.tile_pool(name="dram", bufs=2, space="DRAM") as dram_pool,
            tc.tile_pool(name="s_ps", bufs=2, space="PSUM") as s_pool,
            tc.tile_pool(name="o_ps", bufs=1, space="PSUM") as o_pool,
            tc.tile_pool(name="l_ps", bufs=1, space="PSUM") as l_pool,
        ):
            ones_f = const_pool.tile([P, 1], f32, tag="ones_f")
            nc.vector.memset(ones_f[:], 1.0)
            ones_r = const_pool.tile([P, 1], f32r, tag="ones_r")
            nc.vector.tensor_copy(ones_r[:], ones_f[:])

            inputs = {}
            NB = BPC * repeat

            def load_batch(bi):
                b = bi % BPC
                qt = in_pool.tile([P, S], f32r, tag="qt")
                kt = in_pool.tile([P, S], f32r, tag="kt")
                v_r = in_pool.tile([P, NJ, D], f32r, tag="v_r")
                if HWDGE_LOADS:
                    # fast plain loads + DVE rounding pass to fp32r
                    qf = in_pool.tile([P, S], f32, tag="qf")
                    kf = in_pool.tile([P, S], f32, tag="kf")
                    vf = in_pool.tile([P, NJ, D], f32, tag="vf")
                    v_src = Vd[b].rearrange("(n p) d -> p n d", p=P)
                    nc.sync.dma_start(kf[:, :256], Ktd[b, :, :256])
                    nc.sync.dma_start(qf[:, :QH], Qtd[b, :, :QH])
                    nc.sync.dma_start(kf[:, 256:], Ktd[b, :, 256:])
                    nc.vector.tensor_copy(kt[:, :256], kf[:, :256])
                    nc.vector.tensor_copy(qt[:, :QH], qf[:, :QH])
                    nc.sync.dma_start(vf[:, :NJ // 2], v_src[:, :NJ // 2])
                    nc.vector.tensor_copy(kt[:, 256:], kf[:, 256:])
                    nc.sync.dma_start(qf[:, QH:], Qtd[b, :, QH:])
                    nc.vector.tensor_copy(v_r[:, :NJ // 2], vf[:, :NJ // 2])
                    nc.sync.dma_start(vf[:, NJ // 2:], v_src[:, NJ // 2:])
                    nc.vector.tensor_copy(qt[:, QH:], qf[:, QH:])
                    nc.vector.tensor_copy(v_r[:, NJ // 2:], vf[:, NJ // 2:])
                else:
                    # SWDGE casting DMA rounds fp32 -> fp32r on the way in;
                    # head chunks first so compute can start early.
                    nc.gpsimd.dma_start(kt[:, :256], Ktd[b, :, :256])
                    nc.gpsimd.dma_start(qt[:, :QH], Qtd[b, :, :QH])
                    v_src = Vd[b].rearrange("(n p) d -> p n d", p=P)
                    nc.gpsimd.dma_start(v_r[:, :NJ // 2], v_src[:, :NJ // 2])
                    nc.gpsimd.dma_start(kt[:, 256:], Ktd[b, :, 256:])
                    nc.gpsimd.dma_start(qt[:, QH:], Qtd[b, :, QH:])
                    nc.gpsimd.dma_start(v_r[:, NJ // 2:], v_src[:, NJ // 2:])
                inputs[bi] = (qt, kt, v_r)

            load_batch(0)

            iters = [
                (bi, h, j)
                for bi in range(NB)
                for h in range(NPH)
                for j in range(NJ)
            ]
            T = len(iters)

            def emit_scores(t):
                bi, h, j = iters[t]
                qt, kt, _ = inputs[bi]
                s_ps = s_pool.tile([P, QH], f32, tag="s")
                for c in range(QH // NC_):
                    nc.tensor.matmul(
                        s_ps[:, c * NC_:(c + 1) * NC_],
                        kt[:, j * P:(j + 1) * P],
                        qt[:, h * QH + c * NC_: h * QH + (c + 1) * NC_],
                        start=True, stop=True,
                    )
                return s_ps

            s_next = emit_scores(0)
            l_ps = o_ps = None
            for t in range(T):
                bi, h, j = iters[t]
                b = bi % BPC
                if j == 0:
                    l_ps = l_pool.tile([1, QH], f32, tag="l")
                    o_ps = o_pool.tile([P, QH], f32, tag="o")
                s_ps = s_next
                pt = pt_pool.tile([P, QH], f32r, tag="pt")
                nc.scalar.activation(pt[:], s_ps[:], EXP, scale=SCALE)
                # prefetch next batch's inputs halfway through this batch
                if h == 1 and j == 0 and bi + 1 < NB:
                    load_batch(bi + 1)
                # software pipeline: issue the next scores matmuls ahead of
                # this iteration's PSUM-consumers so the in-order PE never
                # stalls on the ACT result.
                if t + 1 < T:
                    s_next = emit_scores(t + 1)
                for c in range(QH // NC_):
                    nc.tensor.matmul(
                        l_ps[:, c * NC_:(c + 1) * NC_],
                        ones_r[:],
                        pt[:, c * NC_:(c + 1) * NC_],
                        start=(j == 0), stop=(j == NJ - 1),
                    )
                    nc.tensor.matmul(
                        o_ps[:, c * NC_:(c + 1) * NC_],
                        inputs[bi][2][:, j, :],
                        pt[:, c * NC_:(c + 1) * NC_],
                        start=(j == 0), stop=(j == NJ - 1),
                    )
                if j == NJ - 1:
                    # drain: free the PSUM accumulators quickly, then
                    # normalize out of SBUF.
                    o_sb = ot_pool.tile([P, QH], f32, tag="o_sb")
                    nc.vector.tensor_copy(o_sb[:], o_ps[:])
                    recip = misc_pool.tile([1, QH], f32, tag="recip")
                    nc.vector.reciprocal_approx_fast(recip[:], l_ps[:])
                    # broadcast 1/l across partitions via a DRAM round-trip
                    # (stride-0 partition reads are not allowed from SBUF)
                    rdram = dram_pool.tile([1, QH], f32, tag="rdram")
                    nc.sync.dma_start(rdram[:], recip[:])
                    bcast = misc_pool.tile([P, QH], f32, tag="bcast")
                    nc.sync.dma_start(
                        bcast[:], rdram[0][None].to_broadcast((P, QH))
                    )
                    ot = ot_pool.tile([P, QH], f32, tag="ot")
                    nc.vector.tensor_mul(ot[:], o_sb[:], bcast[:])
                    nc.sync.dma_start(Otd[b, :, h * QH:(h + 1) * QH], ot[:])

    nc.compile()
    return nc


_nc_cache = None


def _get_nc():
    global _nc_cache
    if _nc_cache is None:
        _nc_cache = build()
    return _nc_cache


def kernel(Q_p, K_p, V_p, trace=False):
    Q_p = np.asarray(Q_p, dtype=np.float32)
    K_p = np.asarray(K_p, dtype=np.float32)
    V_p = np.asarray(V_p, dtype=np.float32)
    Qt = np.ascontiguousarray(Q_p.transpose(0, 2, 1))  # [B, D, S]
    Kt = np.ascontiguousarray(K_p.transpose(0, 2, 1))
    nc = _get_nc()
    in_maps = [
        {
            "Qt": Qt[c * BPC:(c + 1) * BPC],
            "Kt": Kt[c * BPC:(c + 1) * BPC],
            "V_p": V_p[c * BPC:(c + 1) * BPC],
        }
        for c in range(N_CORES)
    ]
    res = bass_utils.run_bass_kernel_spmd(
        nc, in_maps, core_ids=list(range(N_CORES)), trace=trace
    )
    out = np.empty((B, S, D), dtype=np.float32)
    for c in range(N_CORES):
        ot = res.results[c]["Ot"]  # [BPC, D, S]
        out[c * BPC:(c + 1) * BPC] = np.ascontiguousarray(ot.transpose(0, 2, 1))
    if trace:
        kernel.last_exec_time_ns = res.exec_time_ns
        kernel.last_results = res
    return out
